# revision 11
# baseline (speedup 1.0000x reference)
"""Bahdanau attention kernel for Trainium2 (8 NeuronCores, data-parallel over batch).

Reference computation (per batch row b):
    pq      = query @ Wq.T                       # (B, AD)
    hidden  = tanh(pq[:, None, :] + processed_memory)   # (B, T, AD)
    e       = einsum('btd,d->bt', hidden, v)     # (B, T)
    e       = where(mask, -1e30, e)
    out     = softmax(e, axis=1)

Device strategy (per core, 8 batches):
  * processed_memory is host-transposed to [b, AD, T] so AD sits on SBUF
    partitions.  The per-d "+pq" add then folds into the ScalarE tanh as a
    per-partition activation bias (free), and the v-weighted reduction over d
    becomes TensorE matmuls with a [128,1] stationary v column (M=1, free up
    to 512) accumulating in PSUM.
  * Energies strips [1, 2048] leave PSUM via a VectorE copy, then tiny
    SBUF->SBUF DMAs relayout them into an [8, T] tile (one batch per
    partition) where the masked softmax runs along the free dimension:
    exp on ScalarE, mask-multiply + row-sum fused in one
    tensor_tensor_reduce, reciprocal + scale on VectorE.
  * mask is applied multiplicatively: softmax(where(m,-1e30,e)) ==
    exp(e)*(1-m) / sum(exp(e)*(1-m)) exactly (exp(-1e30) underflows to 0,
    and |e| <= sum|v| ~ 13 so exp(e) cannot overflow in fp32).
"""

import sys

if "/opt/trn_rl_repo" not in sys.path:
    sys.path.insert(0, "/opt/trn_rl_repo")

import numpy as np

import concourse.bacc as bacc
import concourse.bass as bass
import concourse.tile as tile
from concourse import mybir
from concourse.bass_utils import run_bass_kernel_spmd

B, T, QD, AD = 64, 4096, 1024, 256
NCORES = 8
BLOC = B // NCORES  # batches per core
KB = QD // 128      # k-blocks for the pq matmul
DB = AD // 128      # d-blocks (partition blocks of AD)
F32 = mybir.dt.float32
U8 = mybir.dt.uint8


def build_nc() -> bass.Bass:
    # Bacc (not plain Bass): its nop/event-semaphore lowering passes are what
    # let Tile-scheduled instructions carry multiple semaphore waits.
    nc = bacc.Bacc(None, target_bir_lowering=False)

    pm_t = nc.declare_dram_parameter("pm_t", [BLOC, AD, T], F32, isOutput=False)
    # qT[p, kb*BLOC + b] = query[b, kb*128 + p]  (host-packed, partition-major)
    qT = nc.declare_dram_parameter("qT", [128, KB * BLOC], F32, isOutput=False)
    msk = nc.declare_dram_parameter("mask", [BLOC, T], U8, isOutput=False)
    WqT = nc.declare_dram_parameter("WqT", [QD, AD], F32, isOutput=False)
    v_r = nc.declare_dram_parameter("v_r", [128, DB], F32, isOutput=False)
    out = nc.declare_dram_parameter("out", [BLOC, T], F32, isOutput=True)

    Tanh = mybir.ActivationFunctionType.Tanh
    Exp = mybir.ActivationFunctionType.Exp
    mult = mybir.AluOpType.mult
    add = mybir.AluOpType.add

    HT = 2048          # energies strip length (4 PSUM banks)
    NMM = HT // 512    # matmuls per strip per d-block

    with tile.TileContext(nc) as tc:
        with (
            tc.tile_pool(name="singles", bufs=1) as singles,
            tc.tile_pool(name="pm", bufs=3) as pm_pool,
            tc.tile_pool(name="hid", bufs=3) as hid_pool,
            tc.tile_pool(name="estrip", bufs=2) as estrip_pool,
            tc.tile_pool(name="epsum", bufs=2, space="PSUM") as epsum_pool,
        ):
            # ---- constant loads ----
            wq_sb = singles.tile([128, KB, AD], F32)
            nc.sync.dma_start(
                out=wq_sb, in_=WqT[:, :].rearrange("(kb p) d -> p kb d", p=128)
            )
            qt_sb = singles.tile([128, KB, BLOC], F32)
            nc.sync.dma_start(
                out=qt_sb, in_=qT[:, :].rearrange("p (kb b) -> p kb b", b=BLOC)
            )
            v_sb = singles.tile([128, DB], F32)
            nc.sync.dma_start(out=v_sb, in_=v_r[:, :])
            mask_sb = singles.tile([BLOC, T], U8)
            nc.sync.dma_start(out=mask_sb, in_=msk[:, :])

            # maskz = 1 - mask (1.0 where kept), cast u8 -> f32 in-op
            maskz_sb = singles.tile([BLOC, T], F32)
            nc.vector.tensor_scalar(
                out=maskz_sb,
                in0=mask_sb,
                scalar1=-1.0,
                scalar2=1.0,
                op0=mult,
                op1=add,
            )

            # ---- pq = Wq @ query.T, laid out [d % 128, dblk, b] ----
            pq_sb = singles.tile([128, DB, BLOC], F32)
            for d in range(DB):
                ppq = epsum_pool.tile([128, BLOC], F32, tag="ep")
                for k in range(KB):
                    nc.tensor.matmul(
                        ppq,
                        lhsT=wq_sb[:, k, d * 128 : (d + 1) * 128],
                        rhs=qt_sb[:, k, :],
                        start=(k == 0),
                        stop=(k == KB - 1),
                    )
                nc.scalar.copy(pq_sb[:, d, :], ppq)

            e_sb = singles.tile([BLOC, T], F32)
            work = singles.tile([BLOC, T], F32)
            rowsum = singles.tile([BLOC, 1], F32)
            rinv = singles.tile([BLOC, 1], F32)

            # ---- main loop: tanh + v-reduction ----
            for b in range(BLOC):
                hid = []
                for d in range(DB):
                    pm_sb = pm_pool.tile([128, T], F32)
                    nc.sync.dma_start(
                        out=pm_sb, in_=pm_t[b, d * 128 : (d + 1) * 128, :]
                    )
                    h = hid_pool.tile([128, T], F32)
                    nc.scalar.activation(
                        out=h,
                        in_=pm_sb,
                        func=Tanh,
                        bias=pq_sb[:, d, b : b + 1],
                        scale=1.0,
                    )
                    hid.append(h)
                for half in range(T // HT):
                    ep = epsum_pool.tile([1, HT], F32, tag="ep")
                    for c in range(NMM):
                        lo = half * HT + c * 512
                        nc.tensor.matmul(
                            ep[:, c * 512 : (c + 1) * 512],
                            lhsT=v_sb[:, 0:1],
                            rhs=hid[0][:, lo : lo + 512],
                            start=True,
                            stop=False,
                        )
                        nc.tensor.matmul(
                            ep[:, c * 512 : (c + 1) * 512],
                            lhsT=v_sb[:, 1:2],
                            rhs=hid[1][:, lo : lo + 512],
                            start=False,
                            stop=True,
                        )
                    es = estrip_pool.tile([1, HT], F32)
                    nc.vector.tensor_copy(out=es, in_=ep)
                    nc.gpsimd.dma_start(
                        out=e_sb[b : b + 1, half * HT : (half + 1) * HT], in_=es
                    )

            # ---- masked softmax along free dim (batch per partition) ----
            # (compute-engine APs must start at partition 0/32/64/96, so one
            # pass over all 8 batch-partitions)
            nc.scalar.activation(out=work, in_=e_sb, func=Exp)
            # (tensor_tensor_reduce is a custom ant-dve ucode op that faults
            # on this runtime — use the two standard ops instead)
            nc.vector.tensor_mul(work, work, maskz_sb)
            nc.vector.reduce_sum(out=rowsum, in_=work, axis=mybir.AxisListType.X)
            nc.vector.reciprocal(out=rinv, in_=rowsum)
            nc.vector.tensor_scalar_mul(out=work, in0=work, scalar1=rinv)
            nc.sync.dma_start(out=out[:, :], in_=work)

    # Run the Bacc lowering passes (move_matmul_waits_to_ldweights,
    # generate_event_semaphores, alloc_regs, ...) — run_bass_via_pjrt takes
    # the module as-is and walrus rejects unlowered multi-wait instructions.
    nc.finalize()
    return nc


_CACHE: dict = {}


def _get_nc() -> bass.Bass:
    if "nc" not in _CACHE:
        _CACHE["nc"] = build_nc()
    return _CACHE["nc"]


def make_in_maps(query, processed_memory, mask, Wq, v):
    query = np.ascontiguousarray(np.asarray(query, dtype=np.float32))
    pm = np.asarray(processed_memory, dtype=np.float32)
    mask_u8 = np.asarray(mask).astype(np.uint8)
    Wq = np.asarray(Wq, dtype=np.float32)
    v = np.asarray(v, dtype=np.float32)

    WqT = np.ascontiguousarray(Wq.T)                  # (QD, AD)
    v_r = np.ascontiguousarray(v.reshape(DB, 128).T)  # (128, DB)

    in_maps = []
    for i in range(NCORES):
        sl = slice(i * BLOC, (i + 1) * BLOC)
        in_maps.append(
            {
                "pm_t": np.ascontiguousarray(pm[sl].transpose(0, 2, 1)),
                "qT": np.ascontiguousarray(
                    query[sl]
                    .T.reshape(KB, 128, BLOC)
                    .transpose(1, 0, 2)
                    .reshape(128, KB * BLOC)
                ),
                "mask": np.ascontiguousarray(mask_u8[sl]),
                "WqT": WqT,
                "v_r": v_r,
            }
        )
    return in_maps


def run_spmd(in_maps, **kwargs):
    return run_bass_kernel_spmd(_get_nc(), in_maps, list(range(NCORES)), **kwargs)


def kernel(query, processed_memory, mask, Wq, v) -> np.ndarray:
    in_maps = make_in_maps(query, processed_memory, mask, Wq, v)
    res = run_spmd(in_maps)
    return np.concatenate(
        [res.results[i]["out"] for i in range(NCORES)], axis=0
    ).astype(np.float32)


# revision 16
# speedup vs baseline: 1.2653x; 1.2653x over previous
"""Bahdanau attention kernel for Trainium2 (8 NeuronCores, data-parallel over batch).

Reference computation (per batch row b):
    pq      = query @ Wq.T                       # (B, AD)
    hidden  = tanh(pq[:, None, :] + processed_memory)   # (B, T, AD)
    e       = einsum('btd,d->bt', hidden, v)     # (B, T)
    e       = where(mask, -1e30, e)
    out     = softmax(e, axis=1)

Device strategy (per core, 8 batches):
  * processed_memory is host-transposed to [b, AD, T] so AD sits on SBUF
    partitions.  The per-d "+pq" add then folds into the ScalarE tanh as a
    per-partition activation bias (free), and the v-weighted reduction over d
    becomes TensorE matmuls with a [128,1] stationary v column (M=1, free up
    to 512) accumulating in PSUM.
  * Energies strips [1, 2048] leave PSUM via a VectorE copy, then tiny
    SBUF->SBUF DMAs relayout them into an [8, T] tile (one batch per
    partition) where the masked softmax runs along the free dimension:
    exp on ScalarE, mask-multiply + row-sum fused in one
    tensor_tensor_reduce, reciprocal + scale on VectorE.
  * mask is applied multiplicatively: softmax(where(m,-1e30,e)) ==
    exp(e)*(1-m) / sum(exp(e)*(1-m)) exactly (exp(-1e30) underflows to 0,
    and |e| <= sum|v| ~ 13 so exp(e) cannot overflow in fp32).
"""

import sys

if "/opt/trn_rl_repo" not in sys.path:
    sys.path.insert(0, "/opt/trn_rl_repo")

import numpy as np

import concourse.bacc as bacc
import concourse.bass as bass
import concourse.tile as tile
from concourse import mybir
from concourse.bass_utils import run_bass_kernel_spmd

B, T, QD, AD = 64, 4096, 1024, 256
NCORES = 8
BLOC = B // NCORES  # batches per core
KB = QD // 128      # k-blocks for the pq matmul
DB = AD // 128      # d-blocks (partition blocks of AD)
F32 = mybir.dt.float32
F16 = mybir.dt.float16
U8 = mybir.dt.uint8


def build_nc() -> bass.Bass:
    # Bacc (not plain Bass): its nop/event-semaphore lowering passes are what
    # let Tile-scheduled instructions carry multiple semaphore waits.
    nc = bacc.Bacc(None, target_bir_lowering=False)

    pm_t = nc.declare_dram_parameter("pm_t", [BLOC, AD, T], F32, isOutput=False)
    # qT[p, kb*BLOC + b] = query[b, kb*128 + p]  (host-packed, partition-major)
    qT = nc.declare_dram_parameter("qT", [128, KB * BLOC], F32, isOutput=False)
    msk = nc.declare_dram_parameter("mask", [BLOC, T], U8, isOutput=False)
    WqT = nc.declare_dram_parameter("WqT", [QD, AD], F32, isOutput=False)
    v_r = nc.declare_dram_parameter("v_r", [128, DB], F32, isOutput=False)
    out = nc.declare_dram_parameter("out", [BLOC, T], F32, isOutput=True)

    Tanh = mybir.ActivationFunctionType.Tanh
    Exp = mybir.ActivationFunctionType.Exp
    mult = mybir.AluOpType.mult
    add = mybir.AluOpType.add

    HT = 2048          # energies strip length (4 PSUM banks)
    NMM = HT // 512    # matmuls per strip per d-block

    with tile.TileContext(nc) as tc:
        with (
            tc.tile_pool(name="singles", bufs=1) as singles,
            tc.tile_pool(name="pm", bufs=4) as pm_pool,
            tc.tile_pool(name="hid", bufs=3) as hid_pool,
            tc.tile_pool(name="estrip", bufs=2) as estrip_pool,
            tc.tile_pool(name="epsum", bufs=2, space="PSUM") as epsum_pool,
        ):
            # ---- constant loads ----
            wq_sb = singles.tile([128, KB, AD], F32)
            nc.sync.dma_start(
                out=wq_sb, in_=WqT[:, :].rearrange("(kb p) d -> p kb d", p=128)
            )
            qt_sb = singles.tile([128, KB, BLOC], F32)
            nc.sync.dma_start(
                out=qt_sb, in_=qT[:, :].rearrange("p (kb b) -> p kb b", b=BLOC)
            )
            v_sb = singles.tile([128, DB], F32)
            nc.sync.dma_start(out=v_sb, in_=v_r[:, :])
            # fp16 copy of v for the energies matmuls: fp32 matmuls run as
            # two PE passes at ~4x the cost; tanh outputs are in [-1,1] and
            # v is small, so fp16 (10 mantissa bits) costs ~3e-4 rel err.
            v16_sb = singles.tile([128, DB], F16)
            nc.vector.tensor_copy(out=v16_sb, in_=v_sb)
            mask_sb = singles.tile([BLOC, T], U8)
            nc.sync.dma_start(out=mask_sb, in_=msk[:, :])

            # maskz = 1 - mask (1.0 where kept), cast u8 -> f32 in-op
            maskz_sb = singles.tile([BLOC, T], F32)
            nc.vector.tensor_scalar(
                out=maskz_sb,
                in0=mask_sb,
                scalar1=-1.0,
                scalar2=1.0,
                op0=mult,
                op1=add,
            )

            # ---- pq = Wq @ query.T, laid out [d % 128, dblk, b] ----
            pq_sb = singles.tile([128, DB, BLOC], F32)
            for d in range(DB):
                ppq = epsum_pool.tile([128, BLOC], F32, tag="ep")
                for k in range(KB):
                    nc.tensor.matmul(
                        ppq,
                        lhsT=wq_sb[:, k, d * 128 : (d + 1) * 128],
                        rhs=qt_sb[:, k, :],
                        start=(k == 0),
                        stop=(k == KB - 1),
                    )
                nc.scalar.copy(pq_sb[:, d, :], ppq)

            e_sb = singles.tile([BLOC, T], F32)
            work = singles.tile([BLOC, T], F32)
            rowsum = singles.tile([BLOC, 1], F32)
            rinv = singles.tile([BLOC, 1], F32)

            # ---- main loop: tanh + v-reduction ----
            for b in range(BLOC):
                hid = []
                for d in range(DB):
                    pm_sb = pm_pool.tile([128, T], F32)
                    nc.sync.dma_start(
                        out=pm_sb, in_=pm_t[b, d * 128 : (d + 1) * 128, :]
                    )
                    h = hid_pool.tile([128, T], F16)
                    nc.scalar.activation(
                        out=h,
                        in_=pm_sb,
                        func=Tanh,
                        bias=pq_sb[:, d, b : b + 1],
                        scale=1.0,
                    )
                    hid.append(h)
                for half in range(T // HT):
                    ep = epsum_pool.tile([1, HT], F32, tag="ep")
                    for c in range(NMM):
                        lo = half * HT + c * 512
                        nc.tensor.matmul(
                            ep[:, c * 512 : (c + 1) * 512],
                            lhsT=v16_sb[:, 0:1],
                            rhs=hid[0][:, lo : lo + 512],
                            start=True,
                            stop=False,
                        )
                        nc.tensor.matmul(
                            ep[:, c * 512 : (c + 1) * 512],
                            lhsT=v16_sb[:, 1:2],
                            rhs=hid[1][:, lo : lo + 512],
                            start=False,
                            stop=True,
                        )
                    es = estrip_pool.tile([1, HT], F32)
                    nc.vector.tensor_copy(out=es, in_=ep)
                    nc.gpsimd.dma_start(
                        out=e_sb[b : b + 1, half * HT : (half + 1) * HT], in_=es
                    )

            # ---- masked softmax along free dim (batch per partition) ----
            # (compute-engine APs must start at partition 0/32/64/96, so one
            # pass over all 8 batch-partitions)
            nc.scalar.activation(out=work, in_=e_sb, func=Exp)
            # (tensor_tensor_reduce is a custom ant-dve ucode op that faults
            # on this runtime — use the two standard ops instead)
            nc.vector.tensor_mul(work, work, maskz_sb)
            nc.vector.reduce_sum(out=rowsum, in_=work, axis=mybir.AxisListType.X)
            nc.vector.reciprocal(out=rinv, in_=rowsum)
            nc.vector.tensor_scalar_mul(out=work, in0=work, scalar1=rinv)
            nc.sync.dma_start(out=out[:, :], in_=work)

    # Run the Bacc lowering passes (move_matmul_waits_to_ldweights,
    # generate_event_semaphores, alloc_regs, ...) — run_bass_via_pjrt takes
    # the module as-is and walrus rejects unlowered multi-wait instructions.
    nc.finalize()
    return nc


_CACHE: dict = {}


def _get_nc() -> bass.Bass:
    if "nc" not in _CACHE:
        _CACHE["nc"] = build_nc()
    return _CACHE["nc"]


def make_in_maps(query, processed_memory, mask, Wq, v):
    query = np.ascontiguousarray(np.asarray(query, dtype=np.float32))
    pm = np.asarray(processed_memory, dtype=np.float32)
    mask_u8 = np.asarray(mask).astype(np.uint8)
    Wq = np.asarray(Wq, dtype=np.float32)
    v = np.asarray(v, dtype=np.float32)

    WqT = np.ascontiguousarray(Wq.T)                  # (QD, AD)
    v_r = np.ascontiguousarray(v.reshape(DB, 128).T)  # (128, DB)

    in_maps = []
    for i in range(NCORES):
        sl = slice(i * BLOC, (i + 1) * BLOC)
        in_maps.append(
            {
                "pm_t": np.ascontiguousarray(pm[sl].transpose(0, 2, 1)),
                "qT": np.ascontiguousarray(
                    query[sl]
                    .T.reshape(KB, 128, BLOC)
                    .transpose(1, 0, 2)
                    .reshape(128, KB * BLOC)
                ),
                "mask": np.ascontiguousarray(mask_u8[sl]),
                "WqT": WqT,
                "v_r": v_r,
            }
        )
    return in_maps


def run_spmd(in_maps, **kwargs):
    return run_bass_kernel_spmd(_get_nc(), in_maps, list(range(NCORES)), **kwargs)


def kernel(query, processed_memory, mask, Wq, v) -> np.ndarray:
    in_maps = make_in_maps(query, processed_memory, mask, Wq, v)
    res = run_spmd(in_maps)
    return np.concatenate(
        [res.results[i]["out"] for i in range(NCORES)], axis=0
    ).astype(np.float32)
